# revision 15
# baseline (speedup 1.0000x reference)
"""Trainium2 Bass kernel for nn_CausalSelfAttention (erf-kernel attention).

Sharding: 8 cores = 2 batches x 4 core-groups; each core handles one batch
and 3 of the 12 heads.  Each core computes its 3 heads' attention plus its
partial output projection; the host sums the 4 partials per batch.

v4 design (bf16 storage, fp32 PSUM accumulation):
  - ONE merged instruction stream keeps the PE continuously busy so the
    DVFS p-state ramps to 2.4 GHz: per nt-block emit [QKV(nt), rope(nt),
    vT chunks, attention(si=nt)]; all through one rotating PSUM pool.
  - Host packs q/k weight rows (rope-permuted: even dims then odd dims)
    into wall chunks [q0|q1], [k0|k1], [q2|k2] plus v rows.  v is computed
    directly transposed (lhsT = xT chunk) - no PE transposes.
  - VEXT [128, 16*384]: per chunk c, head h: 128 cols = [v|ones] (h0,h2)
    or [ones|v] (h1).  The ones columns make the AV matmul emit the
    denominator replicated across 64 partitions for free (M=128); h1's
    inverted layout puts its y rows at partitions 64:128 so the
    normalize-mul can write YT01[64:128] lane-aligned, enabling K=128
    head-paired projection matmuls.
  - Scores per (si, t-chunk): [128 t, 512 s], causal chunks only, PAIRED
    into [128,1024] PSUM so erf runs once per pair; band chunks compute
    only the valid column range (causal column reduction).
  - weights = erf(0.125*scores) + 1: ACT erf (the only table resident
    during the stream), DVE +1, Pool affine_select on band chunks.  AV
    matmuls lag LAG pairs behind scores (software pipelining).
  - Unnormalized y+denom copied to SBUF per (h,si); tail phase does
    1/d = exp(-ln(d)) on ACT (2 table loads total), a small SBUF->SBUF
    DMA realigns reciprocal rows across partitions, one DVE mul per head,
    then the head-paired projection, PSUM->SBUF copies, DMA out in fp32.
"""

import os
import sys
from contextlib import ExitStack

import numpy as np

for _p in ("/opt/trn_rl_repo",):
    if _p not in sys.path:
        sys.path.insert(0, _p)

import concourse.bass as bass
import concourse.mybir as mybir
from concourse.bass_utils import run_bass_kernel_spmd
from concourse.tile import TileContext

S = 2048          # sequence length per batch
D = 768           # model dim
HD = 64           # head dim
HPC = 3           # heads per core
NCORES = 8
F32 = mybir.dt.float32
BF16 = mybir.dt.bfloat16
NT = S // 512     # 4 free-dim tiles of 512
TC = S // 128     # 16 t-chunks of 128

# CoreSim doesn't implement Erf; dev-only switch to validate logic in sim.
ERF_FUNC_NAME = "Tanh" if os.environ.get("KERNEL_SIM_TANH", "0") == "1" else "Erf"

LAST_EXEC_NS = None
LAST_RESULTS = None


def _split_multi_waits(nc: bass.Bass) -> None:
    """This walrus build rejects instructions carrying more than one sync
    wait (codegen 'Too many sync wait commands', hit by the Tile kernel-tail
    drain).  Hoist all but the last wait of any multi-wait instruction onto
    single-wait Drain instructions inserted just before it on the same
    engine — semantically identical, one wait per instruction."""
    for f in nc.m.functions:
        for b in f.blocks:
            new_insts = []
            changed = False
            for inst in b.instructions:
                si = inst.sync_info
                waits = list(si.on_wait) if si is not None and si.on_wait else []
                if len(waits) > 1:
                    changed = True
                    for n, w in enumerate(waits[:-1]):
                        d = mybir.InstDrain(
                            name=f"{inst.name}-wsplit{n}",
                            engine=inst.engine,
                            ins=[],
                            outs=[],
                            sync_info=mybir.SyncInfo(on_wait=[w], on_update=[]),
                        )
                        new_insts.append(d)
                    si.on_wait = [waits[-1]]
                new_insts.append(inst)
            if changed:
                b.instructions[:] = new_insts


def build_program() -> bass.Bass:
    nc = bass.Bass(target_bir_lowering=False, debug=False)

    x_t = nc.declare_dram_parameter("xt", [128, 6 * S], BF16, isOutput=False)
    wall = nc.declare_dram_parameter("wall", [128, 6 * 576], BF16, isOutput=False)
    wproj = nc.declare_dram_parameter("wproj", [HPC * HD, D], BF16, isOutput=False)
    csc = nc.declare_dram_parameter("csc", [128, S], BF16, isOutput=False)
    css = nc.declare_dram_parameter("css", [128, S], BF16, isOutput=False)
    swp = nc.declare_dram_parameter("swp", [128, 128], BF16, isOutput=False)
    out_d = nc.declare_dram_parameter("out", [S, D], F32, isOutput=True)

    erf_func = getattr(mybir.ActivationFunctionType, ERF_FUNC_NAME)
    # per-head row split of the AV output: YROW = y rows, DROW = denom rows.
    YROW = [slice(0, HD), slice(HD, 128), slice(0, HD)]
    DROW = [slice(HD, 128), slice(0, HD), slice(HD, 128)]
    OOFF = [HD, 0, HD]     # ones block offset within the head's 128 cols

    with TileContext(nc) as tc:
        with ExitStack() as ctx:
            const = ctx.enter_context(tc.tile_pool(name="const", bufs=1))
            tpool = ctx.enter_context(tc.tile_pool(name="tpool", bufs=3))
            wtp = ctx.enter_context(tc.tile_pool(name="wtp", bufs=5))
            nrm = ctx.enter_context(tc.tile_pool(name="nrm", bufs=2))
            ostp = ctx.enter_context(tc.tile_pool(name="ostp", bufs=3))

            # ---- input DMAs (host pre-packs xt/wall as [128, 6*...] so
            # one descriptor covers all six k-chunks) ----
            WA_all = const.tile([128, 6 * 576], BF16, tag="wa")
            nc.sync.dma_start(out=WA_all, in_=wall[:, :])
            WA = [WA_all[:, kc * 576:(kc + 1) * 576] for kc in range(6)]
            XT_all = const.tile([128, 6 * S], BF16, tag="xt")
            x3 = x_t.rearrange("p (k s) -> p k s", k=6)
            xt3 = XT_all.rearrange("p (k s) -> p k s", k=6)
            ns0 = slice(0, 512)
            nc.sync.dma_start(out=xt3[:, :, ns0], in_=x3[:, :, ns0])
            XT = [XT_all[:, kc * S:(kc + 1) * S] for kc in range(6)]
            SWP = const.tile([128, 128], BF16, tag="swp")
            nc.sync.dma_start(out=SWP, in_=swp[:, :])
            CSC = const.tile([128, S], BF16, tag="csc")
            nc.sync.dma_start(out=CSC, in_=csc[:, :])
            CSS = const.tile([128, S], BF16, tag="css")
            nc.sync.dma_start(out=CSS, in_=css[:, :])
            for ntb in range(1, NT):
                ns = slice(ntb * 512, (ntb + 1) * 512)
                nc.sync.dma_start(out=xt3[:, :, ns], in_=x3[:, :, ns])
            WPP = const.tile([128, D], BF16, tag="wpp")
            nc.sync.dma_start(out=WPP, in_=wproj[0:128, :])
            WP2 = const.tile([HD, D], BF16, tag="wp2")
            nc.sync.dma_start(out=WP2, in_=wproj[128:192, :])

            # VEXT: per chunk c, head h: 128 cols = [v|ones] (h0,h2), [ones|v] (h1)
            VEXT = const.tile([128, TC * 384], BF16, tag="vext")
            v4 = VEXT.rearrange("p (c h x) -> p c h x", c=TC, h=HPC, x=128)
            for h in range(HPC):
                nc.gpsimd.memset(v4[:, :, h, OOFF[h]:OOFF[h] + HD], 1.0)

            C = [const.tile([128, S], BF16, tag=f"c{m}", name=f"c{m}") for m in range(3)]
            R = [const.tile([128, S], BF16, tag=f"r{m}", name=f"r{m}") for m in range(3)]
            # k2 relocated to base partition 0 (matmul requires lhsT/rhs at
            # the same base partition; q2 is at rows 0:64 of R[2])
            K2 = const.tile([HD, S], BF16, tag="k2")
            # heads 0 (rows 0:64) and 1 (rows 64:128) share YT01 so the
            # projection can pair them into K=128 matmuls
            YT01 = const.tile([128, S], BF16, tag="yt01")
            YT2 = const.tile([HD, S], BF16, tag="yt2")
            YU = [const.tile([128, S], F32, tag=f"yu{h}", name=f"yu{h}") for h in range(HPC)]

            QS = [R[0][0:HD, :], R[0][HD:128, :], R[2][0:HD, :]]
            KS = [R[1][0:HD, :], R[1][HD:128, :], K2[:, :]]

            # ---- merged stream: per nt-block [QKV(nt), rope(nt), vT chunks,
            # attention(si=nt)]; AV matmuls lag LAG pairs behind scores ----
            LAG = 3
            with tc.tile_pool(name="mps", bufs=3, space="PSUM") as mps, \
                 tc.tile_pool(name="ypsp", bufs=2, space="PSUM") as ypsp:
                ypsmap = {}
                pend = []

                def emit_av(task, wt):
                    si, h, p, npair = task
                    key = (si, h)
                    if key not in ypsmap:
                        ypsmap[key] = ypsp.tile(
                            [128, 512], F32, tag="yps", name=f"yps{si}_{h}")
                    yps = ypsmap[key]
                    for half in range(2):
                        tcb = 2 * p + half
                        lo = max(0, tcb - 4 * si) * 128
                        nc.tensor.matmul(
                            yps[:, lo:512],
                            lhsT=VEXT[:, tcb * 384 + h * 128:tcb * 384 + (h + 1) * 128],
                            rhs=wt[:, half * 512 + lo:(half + 1) * 512],
                            start=(tcb == 0),
                            stop=(tcb == 2 * npair - 1),
                            skip_group_check=True,
                        )
                    if p == npair - 1:
                        ss = slice(si * 512, (si + 1) * 512)
                        nc.vector.tensor_copy(out=YU[h][:, ss], in_=yps)
                        del ypsmap[key]

                def qk_group(ntb, m):
                    ns = slice(ntb * 512, (ntb + 1) * 512)
                    ps = mps.tile([128, 1024], F32, tag="m", name=f"qk{m}_{ntb}")
                    for kc in range(6):
                        nc.tensor.matmul(
                            ps[:, 0:512],
                            lhsT=WA[kc][:, m * 128:(m + 1) * 128],
                            rhs=XT[kc][:, ns],
                            start=(kc == 0),
                            stop=(kc == 5),
                        )
                    nc.scalar.copy(out=C[m][:, ns], in_=ps[:, 0:512])

                def sw_group(ntb, m):
                    ns = slice(ntb * 512, (ntb + 1) * 512)
                    sw = mps.tile([128, 1024], F32, tag="m", name=f"sw{m}_{ntb}")
                    nc.tensor.matmul(
                        sw[:, 0:512], lhsT=SWP, rhs=C[m][:, ns],
                        start=True, stop=True,
                    )
                    t1 = tpool.tile([128, 512], BF16, tag="t1")
                    t2 = tpool.tile([128, 512], BF16, tag="t2")
                    nc.vector.tensor_mul(t1, C[m][:, ns], CSC[:, ns])
                    nc.vector.tensor_mul(t2, sw[:, 0:512], CSS[:, ns])
                    nc.vector.tensor_add(R[m][:, ns], t1, t2)
                    if m == 2:
                        nc.sync.dma_start(out=K2[:, ns], in_=R[2][HD:128, ns])

                def vt_group(tcb):
                    ts = slice(tcb * 128, (tcb + 1) * 128)
                    ps = mps.tile([128, 1024], F32, tag="m", name=f"vt{tcb}")
                    for kc in range(6):
                        nc.tensor.matmul(
                            ps[:, 0:HPC * HD],
                            lhsT=XT[kc][:, ts],
                            rhs=WA[kc][:, 384:576],
                            start=(kc == 0),
                            stop=(kc == 5),
                        )
                    base = tcb * 384
                    # v0 -> cols [0:64); v1,v2 -> contiguous cols [192:320)
                    nc.scalar.copy(out=VEXT[:, base:base + HD], in_=ps[:, 0:HD])
                    nc.scalar.copy(
                        out=VEXT[:, base + 192:base + 320], in_=ps[:, HD:3 * HD])

                def block_groups(ntb):
                    gs = []
                    for m in range(3):
                        gs.append(lambda m=m: qk_group(ntb, m))
                    for m in range(3):
                        gs.append(lambda m=m: sw_group(ntb, m))
                    for tcb in range(4 * ntb, 4 * ntb + 4):
                        gs.append(lambda tcb=tcb: vt_group(tcb))
                    return gs

                def emit_pair(si, h, p, npair):
                    sc = mps.tile([128, 1024], F32, tag="m", name=f"sc{si}_{h}_{p}")
                    for half in range(2):
                        tcb = 2 * p + half
                        lo = max(0, tcb - 4 * si) * 128
                        nc.tensor.matmul(
                            sc[:, half * 512 + lo:(half + 1) * 512],
                            lhsT=KS[h][:, tcb * 128:(tcb + 1) * 128],
                            rhs=QS[h][:, si * 512 + lo:(si + 1) * 512],
                            start=True,
                            stop=True,
                        )
                    wt = wtp.tile([128, 1024], BF16, tag="wt")
                    if p >= 2 * si:
                        # band pair: erf only the valid column ranges
                        for half in range(2):
                            tcb = 2 * p + half
                            lo = max(0, tcb - 4 * si) * 128
                            nc.scalar.activation(
                                out=wt[:, half * 512 + lo:(half + 1) * 512],
                                in_=sc[:, half * 512 + lo:(half + 1) * 512],
                                func=erf_func, scale=0.125,
                            )
                    else:
                        nc.scalar.activation(
                            out=wt, in_=sc, func=erf_func, scale=0.125)
                    nc.vector.tensor_scalar_add(wt, wt, 1.0)
                    if p >= 2 * si:  # diagonal band: causal mask
                        for half in range(2):
                            tcb = 2 * p + half
                            lo = max(0, tcb - 4 * si) * 128
                            nc.gpsimd.affine_select(
                                out=wt[:, half * 512 + lo:(half + 1) * 512],
                                in_=wt[:, half * 512 + lo:(half + 1) * 512],
                                compare_op=mybir.AluOpType.is_ge,
                                fill=0.0,
                                base=0,
                                channel_multiplier=-1,
                                pattern=[[1, 512 - lo]],
                            )
                    pend.append(((si, h, p, npair), wt))
                    if len(pend) > LAG:
                        emit_av(*pend.pop(0))

                # block 0 up front; block si+1 rationed between attention
                # pairs of si as dependency-free PE filler (keeps the PE
                # queue stocked so the DVFS p-state stays at 2.4 GHz)
                for g in block_groups(0):
                    g()
                for si in range(NT):
                    npair = 2 * (si + 1)
                    fillers = block_groups(si + 1) if si + 1 < NT else []
                    pairs = [(si, h, p, npair) for h in range(HPC) for p in range(npair)]
                    fi = 0
                    for i, (si_, h, p, npair_) in enumerate(pairs):
                        emit_pair(si_, h, p, npair_)
                        want = (i + 1) * len(fillers) // len(pairs)
                        while fi < want:
                            fillers[fi]()
                            fi += 1
                while pend:
                    emit_av(*pend.pop(0))

            # ---- tail: normalize (2 table loads total) + projection ----
            with tc.tile_pool(name="pop", bufs=6, space="PSUM") as pop:
                for h in range(HPC):
                    rs = nrm.tile([128, S], F32, tag="rs", name=f"rs{h}")
                    nc.vector.reciprocal(
                        out=rs[DROW[h], :], in_=YU[h][DROW[h], :],
                    )
                    rl = nrm.tile([128, S], F32, tag="rl", name=f"rl{h}")
                    nc.sync.dma_start(out=rl[YROW[h], :], in_=rs[DROW[h], :])
                    ydst = YT01[YROW[h], :] if h < 2 else YT2[:, :]
                    nc.vector.tensor_mul(ydst, YU[h][YROW[h], :], rl[YROW[h], :])

                for sci in range(TC):
                    scs = slice(sci * 128, (sci + 1) * 128)
                    po1 = pop.tile([128, 512], F32, tag="po")
                    po2 = pop.tile([128, 512], F32, tag="po")
                    nc.tensor.matmul(
                        po1, lhsT=YT01[:, scs], rhs=WPP[:, 0:512],
                        start=True, stop=False,
                    )
                    nc.tensor.matmul(
                        po1, lhsT=YT2[:, scs], rhs=WP2[:, 0:512],
                        start=False, stop=True,
                    )
                    nc.tensor.matmul(
                        po2[:, 0:256], lhsT=YT01[:, scs], rhs=WPP[:, 512:768],
                        start=True, stop=False,
                    )
                    nc.tensor.matmul(
                        po2[:, 0:256], lhsT=YT2[:, scs], rhs=WP2[:, 512:768],
                        start=False, stop=True,
                    )
                    ost = ostp.tile([128, D], F32, tag="ost")
                    if sci % 2 == 0:
                        nc.scalar.copy(out=ost[:, 0:512], in_=po1)
                        nc.vector.tensor_copy(out=ost[:, 512:768], in_=po2[:, 0:256])
                    else:
                        nc.vector.tensor_copy(out=ost[:, 0:512], in_=po1)
                        nc.scalar.copy(out=ost[:, 512:768], in_=po2[:, 0:256])
                    nc.sync.dma_start(out=out_d[scs, :], in_=ost)

    return nc


_PROGRAM = None


def _get_program() -> bass.Bass:
    global _PROGRAM
    if _PROGRAM is None:
        _PROGRAM = build_program()
        _split_multi_waits(_PROGRAM)
    return _PROGRAM


def _bf16(arr):
    return np.ascontiguousarray(arr).astype(mybir.dt.np(BF16))


def make_in_maps(x, Wq, Wk, Wv, Wproj):
    x = np.asarray(x, dtype=np.float32)
    Wq = np.asarray(Wq, dtype=np.float32)
    Wk = np.asarray(Wk, dtype=np.float32)
    Wv = np.asarray(Wv, dtype=np.float32)
    Wproj = np.asarray(Wproj, dtype=np.float32)

    half = HD // 2
    j = np.arange(half, dtype=np.float64)
    freq = 1.0 / (10000.0 ** (j / half))
    ang = np.arange(S, dtype=np.float64)[None, :] * freq[:, None]   # [32, S]
    cosT = np.cos(ang).astype(np.float32)
    sinT = np.sin(ang).astype(np.float32)
    csc = np.tile(np.vstack([cosT, cosT]), (2, 1))                  # [128, S]
    css = np.tile(np.vstack([-sinT, sinT]), (2, 1))

    swp = np.zeros((128, 128), dtype=np.float32)
    for blk in range(2):
        for jj in range(half):
            swp[blk * 64 + jj, blk * 64 + half + jj] = 1.0
            swp[blk * 64 + half + jj, blk * 64 + jj] = 1.0

    perm = np.concatenate([np.arange(0, HD, 2), np.arange(1, HD, 2)])

    in_maps = []
    for c in range(NCORES):
        b = c // 4
        hs = [(c % 4) * HPC + i for i in range(HPC)]
        rq = [Wq[h * HD:(h + 1) * HD][perm, :] for h in hs]
        rk = [Wk[h * HD:(h + 1) * HD][perm, :] for h in hs]
        rv = [Wv[h * HD:(h + 1) * HD, :] for h in hs]
        cols = np.concatenate(
            [rq[0], rq[1], rk[0], rk[1], rq[2], rk[2], rv[0], rv[1], rv[2]],
            axis=0,
        )                                                           # [576, D]
        wallm = np.ascontiguousarray(cols.T)                        # [D, 576]
        dims = np.concatenate([np.arange(h * HD, (h + 1) * HD) for h in hs])
        wproj_t = np.ascontiguousarray(Wproj[:, dims].T)            # [192, D]
        xt = x[b].T                                                 # [D, S]
        xt2 = np.ascontiguousarray(
            xt.reshape(6, 128, S).transpose(1, 0, 2).reshape(128, 6 * S))
        wall2 = np.ascontiguousarray(
            wallm.reshape(6, 128, 576).transpose(1, 0, 2).reshape(128, 6 * 576))
        in_maps.append({
            "xt": _bf16(xt2),
            "wall": _bf16(wall2),
            "wproj": _bf16(wproj_t),
            "csc": _bf16(csc),
            "css": _bf16(css),
            "swp": _bf16(swp),
        })
    return in_maps


def kernel(x, Wq, Wk, Wv, Wproj):
    global LAST_EXEC_NS, LAST_RESULTS
    nc = _get_program()
    in_maps = make_in_maps(x, Wq, Wk, Wv, Wproj)
    trace = os.environ.get("KERNEL_TRACE", "0") == "1"
    res = run_bass_kernel_spmd(nc, in_maps, list(range(NCORES)), trace=trace)
    LAST_EXEC_NS = res.exec_time_ns
    LAST_RESULTS = res
    outs = [np.asarray(r["out"], dtype=np.float32) for r in res.results]
    out = np.empty((2, S, D), dtype=np.float32)
    out[0] = outs[0] + outs[1] + outs[2] + outs[3]
    out[1] = outs[4] + outs[5] + outs[6] + outs[7]
    return out
